# revision 1
# baseline (speedup 1.0000x reference)
"""Trainium2 Bass kernel for nn_CNNModel_82222853915196.

Model (per utterance x: (64, 512)):
  multiscale patch features (h in {8,16,32,64}) -> feats (8192,)
  out[t, :] = Wfc @ concat([x[:, t], feats]) + bfc

Factorization: feats is broadcast over t, so
  out = x.T @ Wfc1.T  +  1 * (Wfc2 @ feats + cconst).T
with Wfc1 = Wfc[:, :64], Wfc2 = Wfc[:, 64:], all feature-bias terms folded
into cconst on the host.

Patch features never materialize an im2col tensor: the patch contraction
  f_h[k,p,o] = sum_{i,j} x[k+i, h*p+j] W_h[k,o,i*h+j]
is computed with "masked" stationary weights over the full 64-row contraction
(rows outside [k, k+h) zeroed host-side), so all offsets k fuse into the
matmul M dim and x is read straight from SBUF with strided APs:
one PSUM-accumulated matmul per within-row offset j.

Weights and feature math run in fp16 (same bytes as bf16, 8x the mantissa);
the frames matmul and final outputs stay fp32. Overall rel err ~4e-4.

Sharding: pure data parallel - 32 utterances -> 8 cores x 4. Weights
replicated; no cross-core communication. DMA issue is spread over the two
HWDGE rings (sync, scalar) + SWDGE (gpsimd) to overlap transfers.
"""

import os
import sys
from contextlib import ExitStack

import numpy as np

for _p in ("/opt/trn_rl_repo", "/root/.axon_site/_ro/trn_rl_repo"):
    if os.path.isdir(_p) and _p not in sys.path:
        sys.path.insert(0, _p)

import concourse.bass as bass
import concourse.tile as tile
from concourse import bacc, mybir
from concourse.bass_utils import run_bass_kernel_spmd

NCORES = 8
NUTT = 4                 # utterances per core
T = 512
F = 64
OUT = 400
W = NUTT * T             # 2048, free width of the x tile
FP32 = mybir.dt.float32
FP16 = mybir.dt.float16
NPF16 = np.float16


# ---------------------------------------------------------------------------
# host-side weight preparation
# ---------------------------------------------------------------------------

def _build_devindex():
    """devindex[kt, fp] = reference flat feature index m in [0, 8192)."""
    devindex = np.full((64, 128), -1, dtype=np.int64)
    # h=8: PSUM (q=k*4+o, u*64+p): kt = p//4, fp = (p%4)*32 + q
    for k in range(8):
        for p in range(64):
            for o in range(4):
                devindex[p // 4, (p % 4) * 32 + k * 4 + o] = (k * 64 + p) * 4 + o
    # h=16: (q=k*16+o, u*32+p): kt = 16 + p//2, fp = (p%2)*64 + q
    for k in range(4):
        for p in range(32):
            for o in range(16):
                devindex[16 + p // 2, (p % 2) * 64 + k * 16 + o] = \
                    2048 + (k * 32 + p) * 16 + o
    # h=32: (q=k*64+o, u*16+p): kt = 32 + p, fp = q
    for k in range(2):
        for p in range(16):
            for o in range(64):
                devindex[32 + p, k * 64 + o] = 4096 + (k * 16 + p) * 64 + o
    # h=64: (u*8+p, o): kt = 48 + p*2 + o//128, fp = o%128
    for p in range(8):
        for o in range(256):
            devindex[48 + p * 2 + o // 128, o % 128] = 6144 + p * 256 + o
    assert devindex.min() >= 0
    return devindex


def _masked(Wh, nk, h, no):
    """w[r, j, k*no+o] = Wh[k, o, (r-k)*h+j] for 0 <= r-k < h else 0."""
    w = np.zeros((64, h, nk * no), dtype=np.float32)
    for k in range(nk):
        for i in range(h):
            w[k + i, :, k * no:(k + 1) * no] = Wh[k].reshape(no, h, h)[:, i, :].T
    return w


def host_prep(W8, b8, W16, b16, W32, b32, W64, b64, Wfc, bfc):
    f32 = np.float32
    W8 = np.asarray(W8, f32); W16 = np.asarray(W16, f32)
    W32 = np.asarray(W32, f32); W64 = np.asarray(W64, f32)
    Wfc = np.asarray(Wfc, f32)
    b8 = np.asarray(b8, f32); b16 = np.asarray(b16, f32)
    b32 = np.asarray(b32, f32); b64 = np.asarray(b64, f32)
    bfc = np.asarray(bfc, f32)

    w8j = _masked(W8, 8, 8, 4).reshape(64, 256)
    w16j = _masked(W16, 4, 16, 16).reshape(64, 1024)
    w32j = _masked(W32, 2, 32, 64).reshape(64, 4096)
    # w64w[i, j*256+o] = W64[o, i*64+j]
    w64w = np.ascontiguousarray(
        W64.reshape(256, 64, 64).transpose(1, 2, 0).reshape(64, 64 * 256))

    devindex = _build_devindex()
    Wfc2 = Wfc[:, 64:]
    wfc2t = np.ascontiguousarray(
        Wfc2[:, devindex.reshape(-1)].T.reshape(64, 128, OUT))
    wfc1t4 = np.ascontiguousarray(np.tile(Wfc[:, :64].T, (1, NUTT)))

    fb = np.zeros(8192, dtype=np.float64)
    fb[0:2048] = np.broadcast_to(b8[:, None, :], (8, 64, 4)).reshape(-1)
    fb[2048:4096] = np.broadcast_to(b16[:, None, :], (4, 32, 16)).reshape(-1)
    fb[4096:6144] = np.broadcast_to(b32[:, None, :], (2, 16, 64)).reshape(-1)
    fb[6144:8192] = np.broadcast_to(b64[None, :], (8, 256)).reshape(-1)
    cconst = (Wfc2.astype(np.float64) @ fb + bfc.astype(np.float64)).astype(f32)

    return {
        "w8j": w8j.astype(NPF16), "w16j": w16j.astype(NPF16),
        "w32j": w32j.astype(NPF16), "w64w": np.ascontiguousarray(w64w.astype(NPF16)),
        "wfc2t": wfc2t.astype(NPF16),
        "wfc1t4": wfc1t4,
        "cconst": np.ascontiguousarray(cconst.reshape(1, OUT) if os.environ.get("K_CC32")
                                       else cconst.reshape(1, OUT).astype(NPF16)),
    }


# ---------------------------------------------------------------------------
# device program
# ---------------------------------------------------------------------------

def build_program(repeat=1, trace_sim=False):
    nc = bacc.Bacc("TRN2", target_bir_lowering=False, debug=False)

    dram = dict(
        x4=nc.dram_tensor("x4", [F, W], FP32, kind="ExternalInput"),
        w8j=nc.dram_tensor("w8j", [64, 256], FP16, kind="ExternalInput"),
        w16j=nc.dram_tensor("w16j", [64, 1024], FP16, kind="ExternalInput"),
        w32j=nc.dram_tensor("w32j", [64, 4096], FP16, kind="ExternalInput"),
        w64w=nc.dram_tensor("w64w", [64, 16384], FP16, kind="ExternalInput"),
        wfc2t=nc.dram_tensor("wfc2t", [64, 128, OUT], FP16, kind="ExternalInput"),
        wfc1t4=nc.dram_tensor("wfc1t4", [64, NUTT * OUT], FP32, kind="ExternalInput"),
        cconst=nc.dram_tensor("cconst", [1, OUT], FP32 if os.environ.get("K_CC32") else FP16, kind="ExternalInput"),
        out=nc.dram_tensor("out", [W, OUT], FP32, kind="ExternalOutput"),
        featsflat=nc.dram_tensor("featsflat", [64, 128, NUTT], FP16),
    )

    with tile.TileContext(nc, trace_sim=trace_sim) as tc:
        for rep in range(repeat):
            with ExitStack() as ctx:
                _emit(nc, tc, ctx, dram, rep)

    nc.compile()
    return nc


def _emit(nc, tc, ctx, dram, rep):
    if os.environ.get("K_ALL_SYNC"):
        class _S:
            dma_start = staticmethod(nc.sync.dma_start)
        scalar_dma = sync_dma = gpsimd_dma = nc.sync.dma_start
    else:
        scalar_dma = nc.scalar.dma_start
        gpsimd_dma = nc.gpsimd.dma_start
        sync_dma = nc.sync.dma_start
    const = ctx.enter_context(tc.tile_pool(name=f"const{rep}", bufs=1))
    stg = ctx.enter_context(tc.tile_pool(name=f"stg{rep}", bufs=2))
    wfc2p = ctx.enter_context(tc.tile_pool(name=f"wfc2p{rep}", bufs=2))
    outp = ctx.enter_context(tc.tile_pool(name=f"outp{rep}", bufs=2))
    ps = ctx.enter_context(tc.tile_pool(name=f"ps{rep}", bufs=2, space="PSUM"))
    psc = ctx.enter_context(tc.tile_pool(name=f"psc{rep}", bufs=1, space="PSUM"))
    psf = ctx.enter_context(tc.tile_pool(name=f"psf{rep}", bufs=2, space="PSUM"))

    CH = 8  # wfc2 k-tiles per streamed chunk

    # ---- input loads. Rings: sync = wfc2 stream; scalar = x4/w64w/out;
    # gpsimd (SWDGE) = small weights, scatters/gathers.
    x4 = const.tile([65, W], FP32, tag="x4")
    scalar_dma(x4[0:64, :], dram["x4"].ap())
    nc.vector.memset(x4[64:65, :], 1.0)
    # fp16 copy of x, duplicated into both 64-partition halves (so operands
    # can sit at base partition 0 or 64 to match w64w's j-parity halves)
    x4h = const.tile([64, W], FP16, tag="x4h")
    nc.vector.tensor_copy(x4h[0:64, :], x4[0:64, :])

    w64w = const.tile([64, 16384], FP16, tag="w64w")
    scalar_dma(w64w[:], dram["w64w"].ap())
    w8j = const.tile([64, 256], FP16, tag="w8j")
    gpsimd_dma(w8j[:], dram["w8j"].ap())
    w16j = const.tile([64, 1024], FP16, tag="w16j")
    gpsimd_dma(w16j[:], dram["w16j"].ap())
    w32j = const.tile([64, 4096], FP16, tag="w32j")
    gpsimd_dma(w32j[:], dram["w32j"].ap())
    cconst = const.tile([1, OUT], FP16 if not os.environ.get("K_CC32") else FP32, tag="cconst")
    gpsimd_dma(cconst[:], dram["cconst"].ap())
    ones1 = const.tile([1, NUTT], FP16 if not os.environ.get("K_CC32") else FP32, tag="ones1")
    nc.vector.memset(ones1[:], 1.0)

    rhs65 = const.tile([65, NUTT * OUT], FP32, tag="rhs65")
    scalar_dma(rhs65[0:64, :], dram["wfc1t4"].ap())

    feats = const.tile([128, 64 * NUTT], FP16, tag="feats")
    cps = psc.tile([NUTT, OUT], FP32, tag="cps")
    featsflat = dram["featsflat"]

    def cmms(b):
        """C matmuls for k-tile block b (16 kts = 2 chunks of CH)."""
        for ch in (2 * b, 2 * b + 1):
            chunk = wfc2p.tile([128, CH * OUT], FP16, tag="wfc2chunk")
            sync_dma(
                chunk[:],
                bass.AP(tensor=dram["wfc2t"], offset=ch * CH * 128 * OUT,
                        ap=[[OUT, 128], [128 * OUT, CH], [1, OUT]]))
            for i in range(CH):
                kt = ch * CH + i
                nc.tensor.matmul(cps[:], feats[:, kt * NUTT:(kt + 1) * NUTT],
                                 chunk[:, i * OUT:(i + 1) * OUT],
                                 start=(kt == 0), stop=False)

    def gather(b):
        gpsimd_dma(
            feats[:, b * 16 * NUTT:(b + 1) * 16 * NUTT],
            bass.AP(tensor=featsflat, offset=b * 16 * 128 * NUTT,
                    ap=[[NUTT, 128], [128 * NUTT, 16], [1, NUTT]]))

    # ---- scale h=8: 8 MMs K=64 M=32 N=256 -> PSUM (k*4+o, u*64+p)
    x8 = x4h[0:64, :].rearrange("i (u p j) -> i u p j", u=NUTT, j=8)
    acc = ps.tile([32, NUTT * 64], FP32, tag="featps")
    for j in range(8):
        nc.tensor.matmul(acc[:], w8j[:, j * 32:(j + 1) * 32], x8[:, :, :, j],
                         start=(j == 0), stop=(j == 7))
    st = stg.tile([32, NUTT * 64], FP16, tag="f8st")
    nc.vector.tensor_copy(st[:], acc[:])
    # scatter (q, u*64+p) -> featsflat[p//4, (p%4)*32+q, u]
    gpsimd_dma(
        bass.AP(tensor=featsflat, offset=0,
                ap=[[NUTT, 32], [1, NUTT], [128 * NUTT, 16], [32 * NUTT, 4]]),
        st[:].rearrange("q (u ph pl) -> q u ph pl", u=NUTT, ph=16))
    gather(0)
    cmms(0)

    # ---- scale h=16: 16 MMs K=64 M=64 N=128 -> PSUM (k*16+o, u*32+p)
    x16 = x4h[0:64, :].rearrange("i (u p j) -> i u p j", u=NUTT, j=16)
    acc = ps.tile([64, NUTT * 32], FP32, tag="featps")
    for j in range(16):
        nc.tensor.matmul(acc[:], w16j[:, j * 64:(j + 1) * 64], x16[:, :, :, j],
                         start=(j == 0), stop=(j == 15))
    st = stg.tile([64, NUTT * 32], FP16, tag="f16st")
    nc.vector.tensor_copy(st[:], acc[:])
    # scatter (q, u*32+p) -> featsflat[16+p//2, (p%2)*64+q, u]
    gpsimd_dma(
        bass.AP(tensor=featsflat, offset=16 * 128 * NUTT,
                ap=[[NUTT, 64], [1, NUTT], [128 * NUTT, 16], [64 * NUTT, 2]]),
        st[:].rearrange("q (u ph pl) -> q u ph pl", u=NUTT, ph=16))
    gather(1)
    cmms(1)

    # ---- scale h=32: 32 MMs K=64 M=128 N=64 -> PSUM (k*64+o, u*16+p)
    x32 = x4h[0:64, :].rearrange("i (u p j) -> i u p j", u=NUTT, j=32)
    acc = ps.tile([128, NUTT * 16], FP32, tag="featps")
    for j in range(32):
        nc.tensor.matmul(acc[:], w32j[:, j * 128:(j + 1) * 128], x32[:, :, :, j],
                         start=(j == 0), stop=(j == 31))
    st = stg.tile([128, NUTT * 16], FP16, tag="f32st")
    nc.vector.tensor_copy(st[:], acc[:])
    # scatter (q, u*16+p) -> featsflat[32+p, q, u]
    gpsimd_dma(
        bass.AP(tensor=featsflat, offset=32 * 128 * NUTT,
                ap=[[NUTT, 128], [1, NUTT], [128 * NUTT, 16]]),
        st[:].rearrange("q (u p) -> q u p", u=NUTT))
    gather(2)
    cmms(2)

    # ---- scale h=64: 64 MMs K=64 M=32 N=256 (x stationary, w64 streamed)
    acc = ps.tile([NUTT * 8, 256], FP32, tag="featps")
    x64 = x4h[0:64, :].rearrange("i (u p j) -> i u p j", u=NUTT, j=64)
    for j in range(64):
        nc.tensor.matmul(acc[:], x64[:, :, :, j],
                         w64w[:, j * 256:(j + 1) * 256],
                         start=(j == 0), stop=(j == 63))
    st = stg.tile([NUTT * 8, 256], FP16, tag="f64st")
    nc.vector.tensor_copy(st[:], acc[:])
    # scatter (u*8+p, o) -> featsflat[48+p*2+o//128, o%128, u]
    for u in range(NUTT):
        gpsimd_dma(
            bass.AP(tensor=featsflat, offset=48 * 128 * NUTT + u,
                    ap=[[2 * 128 * NUTT, 8], [128 * NUTT, 2], [NUTT, 128]]),
            st[u * 8:(u + 1) * 8, :].rearrange("p (g o) -> p g o", g=2))
    gather(3)
    cmms(3)

    # ---- finish C: + cconst, stage, write into rhs65 row 64
    nc.tensor.matmul(cps[:], ones1[:], cconst[:], start=False, stop=True)
    csb = stg.tile([NUTT, OUT], FP32, tag="csb")
    nc.vector.tensor_copy(csb[:], cps[:])
    for u in range(NUTT):
        gpsimd_dma(rhs65[64:65, u * OUT:(u + 1) * OUT], csb[u:u + 1, :])

    # ---- frames matmul: out rows = x^T @ Wfc1^T + 1*(C[u]+cconst)
    for u in range(NUTT):
        fsb = outp.tile([128, 4 * OUT], FP32, tag="framesout")
        for tc_i in range(4):
            fps = psf.tile([128, OUT], FP32, tag="framesps")
            nc.tensor.matmul(
                fps[:],
                x4[:, u * T + tc_i * 128: u * T + (tc_i + 1) * 128],
                rhs65[:, u * OUT:(u + 1) * OUT], start=True, stop=True)
            nc.vector.tensor_copy(fsb[:, tc_i * OUT:(tc_i + 1) * OUT], fps[:])
        scalar_dma(
            bass.AP(tensor=dram["out"], offset=u * T * OUT,
                    ap=[[OUT, 128], [128 * OUT, 4], [1, OUT]]),
            fsb[:])


_NC_CACHE = None


def _get_nc():
    global _NC_CACHE
    if _NC_CACHE is None:
        _NC_CACHE = build_program()
    return _NC_CACHE


# ---------------------------------------------------------------------------
# entry point
# ---------------------------------------------------------------------------

def run(inputs, trace=False, **kw):
    nc = _get_nc()
    prep = host_prep(inputs["W8"], inputs["b8"], inputs["W16"], inputs["b16"],
                     inputs["W32"], inputs["b32"], inputs["W64"], inputs["b64"],
                     inputs["Wfc"], inputs["bfc"])
    batch = np.asarray(inputs["batch"], np.float32)
    in_maps = []
    for c in range(NCORES):
        x4 = np.ascontiguousarray(
            batch[NUTT * c:NUTT * (c + 1)].transpose(1, 0, 2).reshape(F, W))
        m = dict(prep)
        m["x4"] = x4
        in_maps.append(m)
    res = run_bass_kernel_spmd(nc, in_maps, core_ids=list(range(NCORES)),
                               trace=trace, **kw)
    out = np.concatenate([r["out"] for r in res.results], axis=0)
    return out, res


def kernel(**inputs):
    out, _ = run(inputs)
    return out



# revision 6
# speedup vs baseline: 3.8216x; 3.8216x over previous
"""Trainium2 Bass kernel for nn_CNNModel_82222853915196.

Model (per utterance x: (64, 512)):
  multiscale patch features (h in {8,16,32,64}) -> feats (8192,)
  out[t, :] = Wfc @ concat([x[:, t], feats]) + bfc

Factorization: feats is broadcast over t, so
  out = x.T @ Wfc1.T + 1 * (Wfc2 @ feats + cconst).T
with all feature-bias terms folded into cconst on the host.

v2 design (vs the HBM-scatter baseline): everything stays on-chip.
The masked-stationary-weight patch matmuls are restructured so each
scale's PSUM tile comes out directly in the layout the Wfc2 contraction
consumes: [f(128 partitions), kt_local*4 + u]. Tricks:
  - p%4 / p%2 column-parity of a patch index is routed to PSUM partition
    BANDS by splitting the j-offset loop per parity (out partition base
    pl*32 / pl*64), so no cross-partition shuffle is ever needed.
  - j-offsets are processed in PAIRS via a second copy of x shifted by
    one column living on partitions 64..127 (K=64 -> K=128).
  - h=64 uses W64 as the stationary operand so output partitions are the
    o%128 feature index directly.
Wfc2 is host-permuted to [f, o_half, kt, o'] so the stream loads are
contiguous 3.2KB-per-partition descriptors, streamed in 16 chunks and
consumed by 128 accumulating matmuls into C[u, o]. The frames matmul
runs transposed (out partitions = o-block) in fp16 with the C row folded
in via a 65th ones-partition; output is written as outT[400, 2048] fp16
and unscrambled on the host.

Sharding: pure data parallel - 32 utterances -> 8 cores x 4; weights
replicated; no cross-core communication.
"""

import os
import sys
from contextlib import ExitStack

import numpy as np

for _p in ("/opt/trn_rl_repo", "/root/.axon_site/_ro/trn_rl_repo"):
    if os.path.isdir(_p) and _p not in sys.path:
        sys.path.insert(0, _p)

import concourse.bass as bass
import concourse.tile as tile
from concourse import bacc, mybir
from concourse.bass_utils import run_bass_kernel_spmd

NCORES = 8
NUTT = 4                 # utterances per core
T = 512
F = 64
OUT = 400
W = NUTT * T             # 2048, free width of the x tile
FP32 = mybir.dt.float32
FP16 = mybir.dt.float16
NPF16 = np.float16


# ---------------------------------------------------------------------------
# host-side weight preparation
# ---------------------------------------------------------------------------

def _build_devindex():
    """devindex[kt, f] = reference flat feature index m in [0, 8192)."""
    devindex = np.full((64, 128), -1, dtype=np.int64)
    # h=8: psF8[f=(p%4)*32 + k*4+o, (p//4)*4+u]
    for k in range(8):
        for p in range(64):
            for o in range(4):
                devindex[p // 4, (p % 4) * 32 + k * 4 + o] = (k * 64 + p) * 4 + o
    # h=16: psF16[f=(p%2)*64 + k*16+o, 16 + p//2]
    for k in range(4):
        for p in range(32):
            for o in range(16):
                devindex[16 + p // 2, (p % 2) * 64 + k * 16 + o] = \
                    2048 + (k * 32 + p) * 16 + o
    # h=32: psF32[f=k*64+o, 32 + p]
    for k in range(2):
        for p in range(16):
            for o in range(64):
                devindex[32 + p, k * 64 + o] = 4096 + (k * 16 + p) * 64 + o
    # h=64: psF64[f=o%128, 48 + p*2 + o//128]
    for p in range(8):
        for o in range(256):
            devindex[48 + p * 2 + o // 128, o % 128] = 6144 + p * 256 + o
    assert devindex.min() >= 0
    return devindex


def _masked(Wh, nk, h, no):
    """w[r, j, k*no+o] = Wh[k, o, (r-k)*h+j] for 0 <= r-k < h else 0."""
    w = np.zeros((64, h, nk * no), dtype=np.float32)
    for k in range(nk):
        for i in range(h):
            w[k + i, :, k * no:(k + 1) * no] = Wh[k].reshape(no, h, h)[:, i, :].T
    return w


def _pair(m):
    """[64, nj, q] -> [128, (nj//2)*q]: row block 0 = even j, block 1 = odd."""
    top = np.ascontiguousarray(m[:, 0::2, :]).reshape(64, -1)
    bot = np.ascontiguousarray(m[:, 1::2, :]).reshape(64, -1)
    return np.concatenate([top, bot], axis=0)


def host_prep(W8, b8, W16, b16, W32, b32, W64, b64, Wfc, bfc):
    f32 = np.float32
    W8 = np.asarray(W8, f32); W16 = np.asarray(W16, f32)
    W32 = np.asarray(W32, f32); W64 = np.asarray(W64, f32)
    Wfc = np.asarray(Wfc, f32)
    b8 = np.asarray(b8, f32); b16 = np.asarray(b16, f32)
    b32 = np.asarray(b32, f32); b64 = np.asarray(b64, f32)
    bfc = np.asarray(bfc, f32)

    w8jj = _pair(_masked(W8, 8, 8, 4))          # [128, 4*32]
    w16jj = _pair(_masked(W16, 4, 16, 16))      # [128, 8*64]
    w32jj = _pair(_masked(W32, 2, 32, 64))      # [128, 16*128]
    # w64ww[i + 64*g, jp*256 + o] = W64[o, i*64 + 2*jp + g]
    w64ww = _pair(W64.reshape(256, 64, 64).transpose(1, 2, 0))  # [128, 32*256]

    devindex = _build_devindex()
    Wfc2 = Wfc[:, 64:]
    # wfc2tf[f, half, kt, o'] = Wfc2[half*200+o', devindex[kt, f]]
    wfc2t = np.ascontiguousarray(
        Wfc2[:, devindex.reshape(-1)].T.reshape(64, 128, OUT))
    wfc2tf = np.ascontiguousarray(
        wfc2t.transpose(1, 0, 2).reshape(128, 64, 2, 200)
        .transpose(0, 2, 1, 3).reshape(128, 64 * OUT))
    wfc1t4 = np.ascontiguousarray(np.tile(Wfc[:, :64].T, (1, NUTT)))  # [64,1600]

    fb = np.zeros(8192, dtype=np.float64)
    fb[0:2048] = np.broadcast_to(b8[:, None, :], (8, 64, 4)).reshape(-1)
    fb[2048:4096] = np.broadcast_to(b16[:, None, :], (4, 32, 16)).reshape(-1)
    fb[4096:6144] = np.broadcast_to(b32[:, None, :], (2, 16, 64)).reshape(-1)
    fb[6144:8192] = np.broadcast_to(b64[None, :], (8, 256)).reshape(-1)
    cconst = (Wfc2.astype(np.float64) @ fb + bfc.astype(np.float64)).astype(f32)

    return {
        "w8jj": w8jj.astype(NPF16), "w16jj": w16jj.astype(NPF16),
        "w32jj": w32jj.astype(NPF16), "w64ww": w64ww.astype(NPF16),
        "wfc2tf": wfc2tf.astype(NPF16),
        "wfc1t4": wfc1t4.astype(NPF16),
        "cconst": np.ascontiguousarray(cconst.reshape(1, OUT)).astype(NPF16),
    }


# ---------------------------------------------------------------------------
# device program
# ---------------------------------------------------------------------------

def build_program(trace_sim=False):
    nc = bacc.Bacc("TRN2", target_bir_lowering=False, debug=False)

    dram = dict(
        xh=nc.dram_tensor("xh", [F, W], FP16, kind="ExternalInput"),
        w8jj=nc.dram_tensor("w8jj", [128, 128], FP16, kind="ExternalInput"),
        w16jj=nc.dram_tensor("w16jj", [128, 512], FP16, kind="ExternalInput"),
        w32jj=nc.dram_tensor("w32jj", [128, 2048], FP16, kind="ExternalInput"),
        w64ww=nc.dram_tensor("w64ww", [128, 8192], FP16, kind="ExternalInput"),
        wfc2tf=nc.dram_tensor("wfc2tf", [128, 64 * OUT], FP16, kind="ExternalInput"),
        wfc1t4=nc.dram_tensor("wfc1t4", [F, NUTT * OUT], FP16, kind="ExternalInput"),
        cconst=nc.dram_tensor("cconst", [1, OUT], FP16, kind="ExternalInput"),
        outT=nc.dram_tensor("outT", [OUT, W], FP16, kind="ExternalOutput"),
    )

    with tile.TileContext(nc, trace_sim=trace_sim) as tc:
        with ExitStack() as ctx:
            _emit(nc, tc, ctx, dram)

    nc.compile()
    return nc


def _emit(nc, tc, ctx, dram):
    scalar_dma = nc.scalar.dma_start
    gpsimd_dma = nc.gpsimd.dma_start
    sync_dma = nc.sync.dma_start

    const = ctx.enter_context(tc.tile_pool(name="const", bufs=1))
    stg = ctx.enter_context(tc.tile_pool(name="stg", bufs=2))
    wfc2p = ctx.enter_context(tc.tile_pool(name="wfc2p", bufs=2))
    outp = ctx.enter_context(tc.tile_pool(name="outp", bufs=2))
    ps = ctx.enter_context(tc.tile_pool(name="ps", bufs=2, space="PSUM"))
    psc = ctx.enter_context(tc.tile_pool(name="psc", bufs=1, space="PSUM"))
    psf = ctx.enter_context(tc.tile_pool(name="psf", bufs=2, space="PSUM"))

    # ---- input loads. Rings: sync = wfc2 stream; scalar = x/w64ww/out;
    # gpsimd (SWDGE) = small weights + C-row bounces.
    # xx: rows 0-63 = x, rows 64-127 = x shifted left one column (j-pairing)
    xx = const.tile([128, W], FP16, tag="xx")
    scalar_dma(xx[0:64, :], dram["xh"].ap())
    scalar_dma(xx[64:128, 0:W - 1], dram["xh"].ap()[:, 1:W])
    nc.vector.memset(xx[64:128, W - 1:W], 0.0)

    w8jj = const.tile([128, 128], FP16, tag="w8jj")
    gpsimd_dma(w8jj[:], dram["w8jj"].ap())
    w16jj = const.tile([128, 512], FP16, tag="w16jj")
    gpsimd_dma(w16jj[:], dram["w16jj"].ap())
    w32jj = const.tile([128, 2048], FP16, tag="w32jj")
    gpsimd_dma(w32jj[:], dram["w32jj"].ap())
    w64ww = const.tile([128, 8192], FP16, tag="w64ww")
    scalar_dma(w64ww[:], dram["w64ww"].ap())
    cconst = const.tile([1, OUT], FP16, tag="cconst")
    gpsimd_dma(cconst[:], dram["cconst"].ap())
    ones1 = const.tile([1, NUTT], FP16, tag="ones1")
    nc.vector.memset(ones1[:], 1.0)

    # frames rhs: rows 0-63 = Wfc1^T tiled per-utt, row 64 = C[u] (bounced in)
    rhs65 = const.tile([65, NUTT * OUT], FP16, tag="rhs65")
    scalar_dma(rhs65[0:64, :], dram["wfc1t4"].ap())

    # frames lhsT: x in fp16 with a 65th ones partition
    x65 = const.tile([65, W], FP16, tag="x65")
    nc.vector.tensor_copy(x65[0:64, :], xx[0:64, :])
    nc.vector.memset(x65[64:65, :], 1.0)

    feats = const.tile([128, 64 * NUTT], FP16, tag="feats")

    # rhs for all masked-scale matmuls: cols (p16, u) at offset j0
    xr = xx[:, :].rearrange("i (u p j) -> i p u j", u=NUTT, p=16, j=32)

    # ---- h=8: psF8[f=(pl*32 + k*4+o), (ph,u)]; bands pl = p%4
    acc = ps.tile([128, 64], FP32, tag="featps")
    for pl in range(4):
        for jp in range(4):
            nc.tensor.matmul(acc[pl * 32:(pl + 1) * 32, :],
                             w8jj[:, jp * 32:(jp + 1) * 32],
                             xr[:, :, :, 8 * pl + 2 * jp],
                             start=(jp == 0), stop=(jp == 3),
                             tile_position=(0, pl * 32))
    nc.vector.tensor_copy(feats[:, 0:64], acc[:])

    # ---- h=16: psF16[f=(pl*64 + k*16+o), (ph,u)]; bands pl = p%2
    acc = ps.tile([128, 64], FP32, tag="featps")
    for pl in range(2):
        for jp in range(8):
            nc.tensor.matmul(acc[pl * 64:(pl + 1) * 64, :],
                             w16jj[:, jp * 64:(jp + 1) * 64],
                             xr[:, :, :, 16 * pl + 2 * jp],
                             start=(jp == 0), stop=(jp == 7),
                             tile_position=(0, pl * 64))
    nc.vector.tensor_copy(feats[:, 64:128], acc[:])

    # ---- h=32: psF32[f=k*64+o, (p,u)]
    acc = ps.tile([128, 64], FP32, tag="featps")
    for jp in range(16):
        nc.tensor.matmul(acc[:],
                         w32jj[:, jp * 128:(jp + 1) * 128],
                         xr[:, :, :, 2 * jp],
                         start=(jp == 0), stop=(jp == 15))
    nc.vector.tensor_copy(feats[:, 128:192], acc[:])

    # ---- h=64: W64 stationary -> psF64[f=o%128, (g=o//128)*32 + (u,p)]
    x64 = xx[:, :].rearrange("i (u p j) -> i u p j", u=NUTT, p=8, j=64)
    acc = ps.tile([128, 64], FP32, tag="featps")
    for g in range(2):
        for jp in range(32):
            nc.tensor.matmul(acc[:, g * 32:(g + 1) * 32],
                             w64ww[:, jp * 256 + g * 128: jp * 256 + (g + 1) * 128],
                             x64[:, :, :, 2 * jp],
                             start=(jp == 0), stop=(jp == 31))
    # feats cols for kt=48+p*2+g, u: 192 + p*8 + g*4 + u  <-  acc[(g,u,p)]
    nc.vector.tensor_copy(
        feats[:, 192:256].rearrange("f (p g u) -> f p g u", p=8, g=2, u=NUTT),
        acc[:].rearrange("f (g u p) -> f p g u", g=2, u=NUTT, p=8))

    # ---- C = Wfc2 @ feats, streamed in 16 chunks (2 o-halves x 8 kt-groups)
    cps = psc.tile([NUTT, OUT], FP32, tag="cps")
    csb = stg.tile([NUTT, OUT], FP16, tag="csb")
    wsrc = dram["wfc2tf"].ap().rearrange("f (c r) -> f c r", c=16)

    for half in range(2):
        for ktg in range(8):
            chunk = wfc2p.tile([128, 8 * 200], FP16, tag="wfc2chunk")
            sync_dma(chunk[:], wsrc[:, half * 8 + ktg, :])
            for i in range(8):
                kt = ktg * 8 + i
                nc.tensor.matmul(cps[:, half * 200:(half + 1) * 200],
                                 feats[:, kt * NUTT:(kt + 1) * NUTT],
                                 chunk[:, i * 200:(i + 1) * 200],
                                 start=(kt == 0), stop=False)
        nc.tensor.matmul(cps[:, half * 200:(half + 1) * 200],
                         ones1[:], cconst[:, half * 200:(half + 1) * 200],
                         start=False, stop=True)
        nc.vector.tensor_copy(csb[:, half * 200:(half + 1) * 200],
                              cps[:, half * 200:(half + 1) * 200])
        for u in range(NUTT):
            gpsimd_dma(
                rhs65[64:65, u * OUT + half * 200: u * OUT + (half + 1) * 200],
                csb[u:u + 1, half * 200:(half + 1) * 200])

        # ---- frames (transposed): psOT[o-block, t] = rhs65-block^T @ x65
        for ob in (2 * half, 2 * half + 1):
            fsb = outp.tile([100, W], FP16, tag="framesout")
            for u in range(NUTT):
                fps = psf.tile([100, T], FP32, tag="framesps")
                nc.tensor.matmul(
                    fps[:],
                    rhs65[:, u * OUT + ob * 100: u * OUT + (ob + 1) * 100],
                    x65[:, u * T:(u + 1) * T], start=True, stop=True)
                eng = nc.vector if u % 2 == 0 else nc.scalar
                if u % 2 == 0:
                    eng.tensor_copy(fsb[:, u * T:(u + 1) * T], fps[:])
                else:
                    eng.copy(fsb[:, u * T:(u + 1) * T], fps[:])
            scalar_dma(
                bass.AP(tensor=dram["outT"], offset=ob * 100 * W,
                        ap=[[W, 100], [1, W]]),
                fsb[:])


_NC_CACHE = None


def _get_nc():
    global _NC_CACHE
    if _NC_CACHE is None:
        _NC_CACHE = build_program()
    return _NC_CACHE


# ---------------------------------------------------------------------------
# entry point
# ---------------------------------------------------------------------------

def run(inputs, trace=False, **kw):
    nc = _get_nc()
    prep = host_prep(inputs["W8"], inputs["b8"], inputs["W16"], inputs["b16"],
                     inputs["W32"], inputs["b32"], inputs["W64"], inputs["b64"],
                     inputs["Wfc"], inputs["bfc"])
    batch = np.asarray(inputs["batch"], np.float32)
    in_maps = []
    for c in range(NCORES):
        xh = np.ascontiguousarray(
            batch[NUTT * c:NUTT * (c + 1)].transpose(1, 0, 2)
            .reshape(F, W).astype(NPF16))
        m = dict(prep)
        m["xh"] = xh
        in_maps.append(m)
    res = run_bass_kernel_spmd(nc, in_maps, core_ids=list(range(NCORES)),
                               trace=trace, **kw)
    outs = []
    for r in res.results:
        o = np.asarray(r["outT"]).astype(np.float32)          # [400, 2048]
        outs.append(o.reshape(OUT, NUTT, T).transpose(1, 2, 0).reshape(-1, OUT))
    return np.concatenate(outs, axis=0), res


def kernel(**inputs):
    out, _ = run(inputs)
    return out
